# revision 30
# baseline (speedup 1.0000x reference)
"""MixedDecoder (dense MoE blend) Trainium2 kernel.

Data-parallel over 8 NeuronCores: batch 512 -> 64 rows/core, expert weights
replicated. All activations kept feature-major ("T" = [feature, batch]) so no
on-device transposes are needed except one tiny [64,8] coeff transpose.

Mixed layer out = sum_e coeff[:,e] * (x @ W_e) + coeff @ b is computed as one
PSUM-accumulated matmul over K = sum_e I rows, where the moving operand X' has
rows x^T scaled per-expert by the gating coefficient (broadcast across
partitions via ones-column outer products on the PE). Weights stream from HBM
in bf16 (~7 MB/core) in a few large DMAs, which is the roofline here.
"""

import numpy as np
import ml_dtypes

import concourse.bass as bass
import concourse.tile as tile
from concourse import bacc, mybir
from concourse import bass_utils

BF16 = mybir.dt.bfloat16
F32 = mybir.dt.float32
AF = mybir.ActivationFunctionType
OP = mybir.AluOpType

B, L, FS, H, E = 512, 64, 96, 512, 8
IN = L + FS          # 160
INTER = L + H        # 576
OUT = FS             # 96
NCORES = 8
BL = B // NCORES     # 64 batch rows per core

_nbf = ml_dtypes.bfloat16

# gpack column layout (bf16 [128, 528]):
_GP_ZCT0 = 0      # [128, 64]  zcT rows 0:128
_GP_ZCT1 = 64     # [32, 64]   zcT rows 128:160
_GP_GW00 = 128    # [128, 64]  gw0 rows 0:128
_GP_GW01 = 192    # [32, 64]   gw0 rows 128:160
_GP_GW1 = 256     # [64, 64]
_GP_GW2 = 320     # [64, 8]
_GP_GB0 = 328     # [1, 64]
_GP_GB1 = 392     # [1, 64]
_GP_GB2 = 456     # [1, 8]
_GP_ID = 464      # [64, 64] identity
_GP_COLS = 528


def _build():
    nc = bacc.Bacc("TRN2", target_bir_lowering=False, debug=False,
                   num_devices=NCORES)

    def din(name, shape, dtype=BF16):
        return nc.dram_tensor(name, list(shape), dtype,
                              kind="ExternalInput").ap()

    gpack = din("gpack", [128, _GP_COLS])
    w0a = din("w0a", [E, 128, 2, 512])     # [e, p, t, o] t in {0,1}; t1 p<32
    w1a = din("w1a", [E, 128, 2048])       # [e, p, t*512+o] h-rows
    w1z = din("w1z", [E, L, 512])
    w2p = din("w2p", [128, E * 480])       # flat-packed w2 (z rows >=64 zero)
    bcat = din("bcat", [E, 1120])          # [b0 | b1 | b2]

    out_d = nc.dram_tensor("out", [BL, OUT], F32, kind="ExternalOutput").ap()

    with tile.TileContext(nc) as tc:
        with (
            tc.tile_pool(name="const", bufs=1) as cpool,
            tc.tile_pool(name="w", bufs=1) as wpool,
            tc.tile_pool(name="x", bufs=1) as xpool,
            tc.tile_pool(name="x12", bufs=2) as x12pool,
            tc.tile_pool(name="act", bufs=2) as apool,
            tc.tile_pool(name="psg", bufs=2, space="PSUM") as psg,
            tc.tile_pool(name="psm", bufs=2, space="PSUM") as psm,
            tc.tile_pool(name="pso", bufs=1, space="PSUM") as pso,
        ):
            # ---- bulk loads: one packed small DMA + few big weight DMAs ----
            gp = cpool.tile([128, _GP_COLS], BF16, tag="gp")
            nc.sync.dma_start(gp[:], gpack[:])

            w0t = wpool.tile([128, E, 1024], BF16, tag="w0")
            nc.sync.dma_start(w0t[:, :, 0:512],
                              w0a[:, :, 0, :].transpose([1, 0, 2]))
            nc.sync.dma_start(w0t[0:32, :, 512:1024],
                              w0a[:, 0:32, 1, :].transpose([1, 0, 2]))

            bc = cpool.tile([E, 1120], BF16, tag="bc")
            nc.sync.dma_start(bc[:], bcat[:])

            w1lo = wpool.tile([128, 4, 2560], BF16, tag="w1lo")
            w1hi = wpool.tile([128, 4, 2560], BF16, tag="w1hi")
            w2t = wpool.tile([128, E, 480], BF16, tag="w2")
            nc.sync.dma_start(w1lo[:, 0:2, 0:2048],
                              w1a[0:2].transpose([1, 0, 2]))
            nc.sync.dma_start(w1lo[:, 2:4, 0:2048],
                              w1a[2:4].transpose([1, 0, 2]))
            nc.sync.dma_start(w1lo[0:L, :, 2048:2560],
                              w1z[0:4].transpose([1, 0, 2]))
            nc.sync.dma_start(w2t[:].rearrange("p a b -> p (a b)"), w2p[:])
            nc.sync.dma_start(w1hi[:, 0:2, 0:2048],
                              w1a[4:6].transpose([1, 0, 2]))
            nc.sync.dma_start(w1hi[0:L, :, 2048:2560],
                              w1z[4:8].transpose([1, 0, 2]))
            nc.sync.dma_start(w1hi[:, 2:4, 0:2048],
                              w1a[6:8].transpose([1, 0, 2]))

            def w1s(e):  # lhsT slice helper for layer-1 weights
                t = w1lo if e < 4 else w1hi
                return t, e % 4

            # gpack views
            zcT0 = gp[:, _GP_ZCT0:_GP_ZCT0 + 64]
            zcT1 = gp[0:32, _GP_ZCT1:_GP_ZCT1 + 64]
            zT = gp[0:L, _GP_ZCT0:_GP_ZCT0 + 64]
            gw00 = gp[:, _GP_GW00:_GP_GW00 + 64]
            gw01 = gp[0:32, _GP_GW01:_GP_GW01 + 64]
            gw1v = gp[0:64, _GP_GW1:_GP_GW1 + 64]
            gw2v = gp[0:64, _GP_GW2:_GP_GW2 + E]
            gb0v = gp[0:1, _GP_GB0:_GP_GB0 + 64]
            gb1v = gp[0:1, _GP_GB1:_GP_GB1 + 64]
            gb2v = gp[0:1, _GP_GB2:_GP_GB2 + E]
            identv = gp[0:64, _GP_ID:_GP_ID + 64]

            ones_t = cpool.tile([1, BL], BF16, tag="ones")
            nc.gpsimd.memset(ones_t[:], 1.0)
            ones128_t = cpool.tile([1, 128], BF16, tag="ones128")
            nc.gpsimd.memset(ones128_t[:], 1.0)

            # ---- ELU: out = exp(min(x,0)) - 1 + relu(x); min/relu run in
            # parallel on DVE/ACT, then exp (ACT), then combine (DVE).
            def elu(dst_bf16, src_psum, shape):
                rl = apool.tile(shape, F32, tag="elu_rl", bufs=4)
                mn = apool.tile(shape, F32, tag="elu_mn", bufs=4)
                ex = apool.tile(shape, F32, tag="elu_ex", bufs=4)
                nc.vector.tensor_scalar_min(mn[:], src_psum, 0.0)
                nc.scalar.activation(rl[:], src_psum, AF.Relu)
                nc.scalar.activation(ex[:], mn[:], AF.Exp)
                nc.vector.scalar_tensor_tensor(dst_bf16, ex[:], -1.0, rl[:],
                                               OP.add, OP.add)

            # ---- gating ----
            g1ps = psg.tile([64, 64], F32, tag="gps", bufs=1)
            nc.tensor.matmul(g1ps[:], gb0v, ones_t[:], start=True, stop=False)
            nc.tensor.matmul(g1ps[:], gw00, zcT0, start=False, stop=False)
            nc.tensor.matmul(g1ps[:], gw01, zcT1, start=False, stop=True)
            g1_t = apool.tile([64, 64], BF16, tag="g1")
            elu(g1_t[:], g1ps[:], [64, 64])

            g2ps = psg.tile([64, 64], F32, tag="gps", bufs=1)
            nc.tensor.matmul(g2ps[:], gb1v, ones_t[:], start=True, stop=False)
            nc.tensor.matmul(g2ps[:], gw1v, g1_t[:], start=False, stop=True)
            g2_t = apool.tile([64, 64], BF16, tag="g2")
            elu(g2_t[:], g2ps[:], [64, 64])

            # logits batch-major [b, e]
            lgps = psg.tile([64, E], F32, tag="gps", bufs=1)
            nc.tensor.matmul(lgps[:], ones_t[:], gb2v, start=True, stop=False)
            nc.tensor.matmul(lgps[:], g2_t[:], gw2v, start=False, stop=True)

            exps_t = apool.tile([64, E], F32, tag="exps")
            se_t = apool.tile([64, 1], F32, tag="se")
            nc.scalar.activation(exps_t[:], lgps[:], AF.Exp, accum_out=se_t[:])
            rec_t = apool.tile([64, 1], F32, tag="rec")
            nc.vector.reciprocal(rec_t[:], se_t[:])
            coeff_t = apool.tile([64, E], BF16, tag="coeff")
            nc.vector.tensor_scalar(coeff_t[:], exps_t[:], rec_t[:], None,
                                    OP.mult)

            # One bf16 PSUM bank holds the coeff transpose [8,64] (cols
            # 0:64) and the 8 per-column row transposes (cols 64:576).
            # S[p, e, b] = coeff[b, e] on all 128 partitions: coeff columns
            # -> partition 0 via per-column PE transposes (no DMA queue
            # dependency), then ones-column outer product per expert.
            misc = psg.tile([E, 576], BF16, tag="misc", bufs=1)
            for e in range(E):
                nc.tensor.matmul(misc[0:1, 64 + 64 * e:128 + 64 * e],
                                 coeff_t[:, e:e + 1], identv,
                                 is_transpose=True, start=True, stop=True)
            nc.tensor.matmul(misc[:, 0:64], coeff_t[:], identv,
                             is_transpose=True, start=True, stop=True)
            coeffT_t = cpool.tile([E, BL], BF16, tag="coeffT")
            nc.vector.tensor_copy(coeffT_t[:], misc[:, 0:64])
            rows_t = cpool.tile([1, E, BL], BF16, tag="rows")
            nc.vector.tensor_copy(rows_t[:].rearrange("p a b -> p (a b)"),
                                  misc[0:1, 64:576])
            S_ps = pso.tile([128, E, BL], F32, tag="S")
            for e in range(E):
                nc.tensor.matmul(S_ps[:, e, :], ones128_t[:],
                                 rows_t[0:1, e, :], start=(e == 0),
                                 stop=(e == E - 1))
            # S in SBUF as bf16: scale ops become all-SBUF 2-byte (faster
            # DVE mode, and gpsimd can read it too)
            S_t = cpool.tile([128, E, BL], BF16, tag="S")
            nc.vector.tensor_copy(S_t[:], S_ps[:])

            # Layers 0/1 run in "layout A": the scaled input X' K-tile is
            # the stationary operand (64 cols) and the weights stream 512
            # wide, so the per-matmul weight-load cost is amortized 8x.
            # h comes out batch-major [64, 512] and is transposed back to
            # feature-major in 4 PE transposes per layer, pipelined per
            # 128-feature chunk with ELU, scaling, and (for L2) matmuls.

            def xscale(xt, hT, t):
                nc.vector.tensor_tensor(
                    xt[:, t, :, :],
                    hT[:, t, :].unsqueeze(1).broadcast_to((128, E, BL)),
                    S_t[:, :, :], OP.mult)

            def xscale_z(xt):
                nc.gpsimd.tensor_tensor(
                    xt[0:L, 4, :, :],
                    zT.unsqueeze(1).broadcast_to((L, E, BL)),
                    S_t[0:L, :, :], OP.mult)

            def seam(l_ps, xt_next):
                """ELU + transpose + rescale, per 128-feature chunk."""
                hb = apool.tile([64, 512], BF16, tag="hb")
                hTp = psm.tile([128, 4, BL], BF16, tag="hTp")
                hT = apool.tile([128, 4, BL], BF16, tag="h")
                for m in range(4):
                    sl = slice(128 * m, 128 * (m + 1))
                    mn = apool.tile([64, 128], F32, tag="elu_mn", bufs=4)
                    rl = apool.tile([64, 128], F32, tag="elu_rl", bufs=4)
                    ex = apool.tile([64, 128], F32, tag="elu_ex", bufs=4)
                    nc.vector.tensor_scalar_min(mn[:], l_ps[:, sl], 0.0)
                    nc.scalar.activation(rl[:], l_ps[:, sl], AF.Relu)
                    nc.scalar.activation(ex[:], mn[:], AF.Exp)
                    nc.vector.scalar_tensor_tensor(hb[:, sl], ex[:], -1.0,
                                                   rl[:], OP.add, OP.add)
                    nc.tensor.matmul(hTp[:, m, :], hb[:, sl], identv,
                                     is_transpose=True, start=True, stop=True)
                    nc.vector.tensor_copy(hT[:, m, :], hTp[:, m, :])
                    xscale(xt_next, hT, m)

            # ---- layer 0 ----
            x0t = xpool.tile([128, 2, E, BL], BF16, tag="x0")
            nc.vector.tensor_tensor(
                x0t[:, 0, :, :],
                zcT0.unsqueeze(1).broadcast_to((128, E, BL)),
                S_t[:, :, :], OP.mult)
            nc.vector.tensor_tensor(
                x0t[0:32, 1, :, :],
                zcT1.unsqueeze(1).broadcast_to((32, E, BL)),
                S_t[0:32, :, :], OP.mult)

            x1t = x12pool.tile([128, 5, E, BL], BF16, tag="x12")
            xscale_z(x1t)
            x2t = x12pool.tile([128, 5, E, BL], BF16, tag="x12")
            xscale_z(x2t)

            l0ps = psm.tile([64, H], F32, tag="lps")
            nc.tensor.matmul(l0ps[:], coeffT_t[:], bc[:, 0:512],
                             start=True, stop=False)
            for e in range(E):
                for t in range(2):
                    K = 128 if t == 0 else 32
                    nc.tensor.matmul(
                        l0ps[:], x0t[0:K, t, e, :],
                        w0t[0:K, e, 512 * t:512 * (t + 1)],
                        start=False, stop=(e == E - 1 and t == 1))
            seam(l0ps, x1t)

            # ---- layer 1 ----
            l1ps = psm.tile([64, H], F32, tag="lps")
            nc.tensor.matmul(l1ps[:], coeffT_t[:], bc[:, 512:1024],
                             start=True, stop=False)
            for e in range(E):
                wt = w1lo if e < 4 else w1hi
                ei = e % 4
                for t in range(5):
                    K = 128 if t < 4 else L
                    nc.tensor.matmul(
                        l1ps[:], x1t[0:K, t, e, :],
                        wt[0:K, ei, 512 * t:512 * (t + 1)],
                        start=False, stop=(e == E - 1 and t == 4))
            seam(l1ps, x2t)

            # ---- layer 2 (t-outer so chunk t only needs h2 chunk t) ----
            l2ps = pso.tile([BL, OUT], F32, tag="ops")
            nc.tensor.matmul(l2ps[:], coeffT_t[:], bc[:, 1024:1120],
                             start=True, stop=False)
            for t in range(4):
                for e in range(E):
                    nc.tensor.matmul(
                        l2ps[:], x2t[:, t, e, :],
                        w2t[:, e, 96 * t:96 * (t + 1)],
                        start=False, stop=False)
            for e in range(E):
                nc.tensor.matmul(
                    l2ps[:], x2t[0:L, 4, e, :], w2t[0:L, e, 384:480],
                    start=False, stop=(e == E - 1))

            out_t = apool.tile([BL, OUT], F32, tag="out_sb")
            nc.vector.tensor_copy(out_t[:], l2ps[:])
            nc.scalar.dma_start(out_d[:], out_t[:])

    nc.compile()
    return nc


_NC_CACHE = None


def _get_nc():
    global _NC_CACHE
    if _NC_CACHE is None:
        _NC_CACHE = _build()
    return _NC_CACHE


def _pack_w2(w2):
    # [128, e*480 + t*96 + o]; t<4 h-rows, t=4 z-rows (partitions >= 64 zero)
    out = np.zeros((128, E * 480), dtype=np.float32)
    hpart = w2[:, L:INTER, :].reshape(E, 4, 128, OUT).transpose(2, 0, 1, 3)
    out.reshape(128, E, 5, OUT)[:, :, 0:4, :] = hpart
    out.reshape(128, E, 5, OUT)[0:L, :, 4, :] = (
        w2[:, 0:L, :].transpose(1, 0, 2))
    return out


def _host_prep(z, c, gw0, gb0, gw1, gb1, gw2, gb2, w0, b0, w1, b1, w2, b2):
    bf = lambda a: np.ascontiguousarray(a).astype(_nbf)
    gp_base = np.zeros((128, _GP_COLS), dtype=np.float32)
    gw0 = np.asarray(gw0)
    gp_base[:, _GP_GW00:_GP_GW00 + 64] = gw0[0:128]
    gp_base[0:32, _GP_GW01:_GP_GW01 + 64] = gw0[128:IN]
    gp_base[0:64, _GP_GW1:_GP_GW1 + 64] = gw1
    gp_base[0:64, _GP_GW2:_GP_GW2 + E] = gw2
    gp_base[0, _GP_GB0:_GP_GB0 + 64] = gb0
    gp_base[0, _GP_GB1:_GP_GB1 + 64] = gb1
    gp_base[0, _GP_GB2:_GP_GB2 + E] = gb2
    gp_base[0:64, _GP_ID:_GP_ID + 64] = np.eye(64, dtype=np.float32)

    w0 = np.asarray(w0)
    w0a = np.zeros((E, 128, 2, 512), dtype=np.float32)
    w0a[:, :, 0, :] = w0[:, 0:128, :]
    w0a[:, 0:32, 1, :] = w0[:, 128:IN, :]
    w1 = np.asarray(w1)
    w2 = np.asarray(w2)
    shared = {
        "w0a": bf(w0a),
        "w1a": bf(w1[:, L:INTER, :].reshape(E, 4, 128, H)
                  .transpose(0, 2, 1, 3).reshape(E, 128, 2048)),
        "w1z": bf(w1[:, 0:L, :]),
        "w2p": bf(_pack_w2(w2)),
        "bcat": bf(np.concatenate([b0, b1, b2], axis=1)),
    }
    zc = np.concatenate([np.asarray(z), np.asarray(c)], axis=1)  # [B, IN]
    in_maps = []
    for i in range(NCORES):
        gpi = gp_base.copy()
        zcT = zc[i * BL:(i + 1) * BL, :].T  # [IN, 64]
        gpi[:, _GP_ZCT0:_GP_ZCT0 + 64] = zcT[0:128]
        gpi[0:32, _GP_ZCT1:_GP_ZCT1 + 64] = zcT[128:IN]
        m = dict(shared)
        m["gpack"] = bf(gpi)
        in_maps.append(m)
    return in_maps


def kernel(**inputs):
    nc = _get_nc()
    in_maps = _host_prep(**inputs)
    res = bass_utils.run_bass_kernel_spmd(nc, in_maps,
                                          core_ids=list(range(NCORES)))
    return np.concatenate([r["out"] for r in res.results], axis=0)
